# revision 44
# baseline (speedup 1.0000x reference)
"""Trainium2 Bass kernel for JointSelfAttention (B=4,T=2048,C=1024,H=16).

Sharding: 8 cores = 4 batches (data-parallel) x 2 head-groups of 8 heads
(tensor-parallel).  Each core computes qkv for its head group, qk-RMSNorm,
RoPE, causal attention, and a partial c_proj; the host sums the two partial
projections per batch and transposes back.

v3: software-pipelined emission.  Attention for query window w only needs
qkv of token tiles <= 4w+3, so attention units (one head x one window) are
interleaved into the qkv producer loop: the Act engine's exp stream (the
critical resource) runs concurrently with qkv GEMMs, norm/rope (DVE) and
projection.  PSUM is the scarce resource (8 banks): q/k/v share one rotating
1-bank accumulator (stats are computed per-tensor), score pairs use 2x2-bank
slots, attn output + transpose staging 1 bank each.  attn@v runs transposed
(stationary 128x128 softmax block, moving v[128x65]) so each block costs 65
PE rows and the softmax denominator lands per-partition.
"""

import math
import numpy as np
from contextlib import ExitStack

B, T, C, H, HD = 4, 2048, 1024, 16, 64
HG = 2              # head groups (tensor-parallel dim)
HPG = H // HG       # heads per group = 8
CG = HPG * HD       # channels per group = 512
N_CORES = B * HG
EPS = float(np.finfo(np.float32).eps)
QW = 512            # query window (free dim per attention block)
NQW = T // QW       # 4 windows
NKT = T // 128      # 16 k tiles
NMT = T // 128      # 16 m (token) tiles
NKC = C // 128      # 8 contraction tiles for qkv


def _split_excess_waits(nc, mybir, max_waits=1):
    """This container's walrus only encodes 1 sync-wait per instruction
    ("Too many sync wait commands" in CoreV3 codegen).  Move extra waits to
    preceding NoOps on the same engine."""
    for f in nc.m.functions:
        for bb in f.blocks:
            new_insts = []
            for inst in bb.instructions:
                si = inst.sync_info
                if si is not None and si.on_wait and len(si.on_wait) > max_waits:
                    waits = list(si.on_wait)
                    extra, keep = waits[:-max_waits], waits[-max_waits:]
                    for i in range(0, len(extra), max_waits):
                        nop = mybir.InstNoOp(
                            name=f"{inst.name}-ws{i}", ins=[], outs=[])
                        nop.engine = inst.engine
                        nop.sync_info = mybir.SyncInfo(
                            on_wait=extra[i:i + max_waits], on_update=[])
                        new_insts.append(nop)
                    inst.sync_info = mybir.SyncInfo(
                        on_wait=keep, on_update=list(si.on_update or []))
                new_insts.append(inst)
            bb.instructions.clear()
            bb.instructions.extend(new_insts)


def _build_nc():
    import concourse.bass as bass
    import concourse.tile as tile
    from concourse import mybir
    from concourse.masks import make_identity

    f32 = mybir.dt.float32
    bf16 = mybir.dt.bfloat16
    AF = mybir.ActivationFunctionType
    MUL = mybir.AluOpType.mult

    nc = bass.Bass("TRN2", debug=False, num_devices=N_CORES)

    xt = nc.dram_tensor("xt", [NMT, 128, NKC * 128], bf16, kind="ExternalInput").ap()
    wqk = nc.dram_tensor("wqk", [C, 2 * CG], bf16, kind="ExternalInput").ap()
    wv = nc.dram_tensor("wv", [C, CG], bf16, kind="ExternalInput").ap()
    wp = nc.dram_tensor("wp", [CG, C], bf16, kind="ExternalInput").ap()
    cosn = nc.dram_tensor("cosn", [128, NMT, HD // 2], bf16, kind="ExternalInput").ap()
    sinn = nc.dram_tensor("sinn", [128, NMT, HD // 2], bf16, kind="ExternalInput").ap()
    trim = nc.dram_tensor("trim", [128, 128], bf16, kind="ExternalInput").ap()
    out = nc.dram_tensor("o", [C, T], f32, kind="ExternalOutput").ap()

    with tile.TileContext(nc) as tc, ExitStack() as ctx:
        # ---- persistent buffers ----
        persist = ctx.enter_context(tc.tile_pool(name="persist", bufs=1))
        # q/k heads transposed: head h lives at partitions (h%2)*64..+64,
        # free slot h//2 -> [128, 4, T]
        qT = persist.tile([128, HPG // 2, T], bf16)
        kT = persist.tile([128, HPG // 2, T], bf16)
        vaug = persist.tile([128, NKT, HPG, HD + 1], bf16)  # v + ones col
        cos_sb = persist.tile([128, NMT, HD // 2], bf16)
        sin_sb = persist.tile([128, NMT, HD // 2], bf16)
        trim_sb = persist.tile([128, 128], bf16)
        ident = persist.tile([128, 128], bf16)
        eps_sb = persist.tile([128, 1], f32)
        ybuf = persist.tile([128, CG // 128, T], bf16)  # attn out (ch x T)
        wqk_sb = persist.tile([128, NKC, 2 * CG], bf16)
        wv_sb = persist.tile([128, NKC, CG], bf16)
        wp_sb = persist.tile([128, CG // 128, C], bf16)
        # window-3 undivided attn results (+denominator col), per (head, qt)
        y3 = persist.tile([128, HPG, 4, HD + 1], bf16)

        # weights ride the Activation HWDGE queue so the first x tile's DMA
        # (SP queue) isn't stuck behind them
        nc.scalar.dma_start(wqk_sb[:, :, 0:CG],
                            wqk[:, 0:CG].rearrange("(kc p) n -> p kc n", p=128))
        nc.scalar.dma_start(wqk_sb[:, :, CG:],
                            wqk[:, CG:].rearrange("(kc p) n -> p kc n", p=128))
        nc.scalar.dma_start(wv_sb[:], wv.rearrange("(kc p) n -> p kc n", p=128))
        nc.scalar.dma_start(cos_sb[:], cosn[:])
        nc.scalar.dma_start(sin_sb[:], sinn[:])
        nc.scalar.dma_start(trim_sb[:], trim[:])
        nc.scalar.dma_start(wp_sb[:], wp.rearrange("(kc p) n -> p kc n", p=128))
        make_identity(nc, ident[:])
        nc.vector.memset(eps_sb[:], EPS)
        nc.gpsimd.memset(
            vaug[:, :, :, HD:HD + 1].rearrange("p a b one -> p (a b one)"), 1.0)

        # ---- work generators, woven at sub-unit granularity ----
        # In-order engines suffer head-of-line blocking: inside an attention
        # unit the PE waits on exp (Act) between score groups.  Emitting qkv
        # GEMM chunks between score groups keeps both engines streaming.
        state = {"mt_done": -1, "win_done": [0] * NQW}

        xpool = ctx.enter_context(tc.tile_pool(name="xp", bufs=3))
        pp1 = ctx.enter_context(tc.tile_pool(name="pp1", bufs=3, space="PSUM"))
        stps = ctx.enter_context(tc.tile_pool(name="stps", bufs=2, space="PSUM"))
        yend = ctx.enter_context(tc.tile_pool(name="yend", bufs=1, space="PSUM"))
        ptpool = ctx.enter_context(tc.tile_pool(name="pt", bufs=3))
        epil = ctx.enter_context(tc.tile_pool(name="epil", bufs=3))
        work = ctx.enter_context(tc.tile_pool(name="work", bufs=3))
        stats = ctx.enter_context(tc.tile_pool(name="stats", bufs=2))
        ostg = ctx.enter_context(tc.tile_pool(name="ostg", bufs=3))

        def p1_gen():
            for mt in range(NMT):
                xt_sb = xpool.tile([128, NKC, 128], bf16, tag="xt")
                nc.sync.dma_start(
                    xt_sb[:].rearrange("p kc t -> p (kc t)"), xt[mt])

                qkn = work.tile([128, 2 * CG], bf16, tag="qkn")
                rr = stats.tile([128, 2 * HPG], f32, tag="rr")

                def stats_norm_rope(sec, s_ps):
                    # rms stats: rr = 1/sqrt(mean(x^2)+eps) per (token, head)
                    sq = work.tile([128, CG], bf16, tag="sq")
                    nc.scalar.activation(sq[:], s_ps[:], AF.Square)
                    ss = stats.tile([128, HPG], f32, tag="ss")
                    nc.vector.tensor_reduce(
                        ss[:], sq[:].rearrange("p (h d) -> p h d", d=HD),
                        axis=mybir.AxisListType.X, op=mybir.AluOpType.add)
                    nc.scalar.activation(rr[:, sec * HPG:(sec + 1) * HPG],
                                         ss[:], AF.Sqrt,
                                         bias=eps_sb[:], scale=1.0 / HD)
                    nc.vector.reciprocal(rr[:, sec * HPG:(sec + 1) * HPG],
                                         rr[:, sec * HPG:(sec + 1) * HPG])
                    # norm multiply doubles as PSUM->SBUF bf16 conversion
                    rr_b = bass.AP(
                        tensor=rr.tensor,
                        offset=rr.offset + sec * HPG,
                        ap=[rr.ap[0], [1, HPG], [0, HD]])
                    qs = qkn[:, sec * CG:(sec + 1) * CG]
                    nc.vector.tensor_tensor(
                        qs.rearrange("p (h d) -> p h d", d=HD),
                        s_ps[:].rearrange("p (h d) -> p h d", d=HD),
                        rr_b, op=MUL)
                    # rope in place: y1 = x1 c + x2 s ; y2 = x2 c - x1 s
                    base = qs.rearrange("p (h two d) -> p h two d",
                                        two=2, d=HD // 2)
                    x1, x2 = base[:, :, 0, :], base[:, :, 1, :]
                    cb = bass.AP(
                        tensor=cos_sb.tensor,
                        offset=cos_sb.offset + mt * (HD // 2),
                        ap=[cos_sb.ap[0], [0, HPG], [1, HD // 2]])
                    sb_ = bass.AP(
                        tensor=sin_sb.tensor,
                        offset=sin_sb.offset + mt * (HD // 2),
                        ap=[sin_sb.ap[0], [0, HPG], [1, HD // 2]])
                    t1 = work.tile([128, HPG, HD // 2], bf16, tag="rt1")
                    t2 = work.tile([128, HPG, HD // 2], bf16, tag="rt2")
                    t3 = work.tile([128, HPG, HD // 2], bf16, tag="rt3")
                    t4 = work.tile([128, HPG, HD // 2], bf16, tag="rt4")
                    nc.gpsimd.tensor_tensor(t1[:], x1, cb, op=MUL)
                    nc.vector.tensor_tensor(t2[:], x2, sb_, op=MUL)
                    nc.gpsimd.tensor_tensor(t3[:], x2, cb, op=MUL)
                    nc.vector.tensor_tensor(t4[:], x1, sb_, op=MUL)
                    nc.vector.tensor_add(x1, t1[:], t2[:])
                    nc.vector.tensor_sub(x2, t3[:], t4[:])

                def transposes(sec, tp, dstT):
                    # head pairs: [128 tok, (2h, 64d)] -> [128, 128]
                    # (partitions 0-63 = head 2j dims, 64-127 = head 2j+1)
                    for j in range(HPG // 2):
                        src = qkn[:, sec * CG + 2 * j * HD:
                                  sec * CG + (2 * j + 2) * HD]
                        nc.tensor.transpose(
                            tp[:, sec, j, :],
                            src.rearrange("p (two d) -> p two d", two=2),
                            ident[:])
                    nc.vector.tensor_copy(
                        dstT[:, :, mt * 128:(mt + 1) * 128], tp[:, sec])

                # q, k, v and the transpose staging rotate through three
                # 1-bank PSUM slots; stats/norm/rope drain while later
                # chunks and woven attention pieces run
                sps = []
                for sec in range(2):
                    s_ps = pp1.tile([128, CG], f32, tag="ps1")
                    sps.append(s_ps)
                    for kc in range(4):
                        nc.tensor.matmul(
                            s_ps[:], xt_sb[:, kc, :],
                            wqk_sb[:, kc, sec * CG:(sec + 1) * CG],
                            start=(kc == 0), stop=False)
                    yield
                    for kc in range(4, NKC):
                        nc.tensor.matmul(
                            s_ps[:], xt_sb[:, kc, :],
                            wqk_sb[:, kc, sec * CG:(sec + 1) * CG],
                            start=False, stop=(kc == NKC - 1))
                    stats_norm_rope(sec, s_ps)
                    yield

                v_ps = pp1.tile([128, CG], f32, tag="ps1")
                for kc in range(4):
                    nc.tensor.matmul(
                        v_ps[:], xt_sb[:, kc, :], wv_sb[:, kc, :],
                        start=(kc == 0), stop=False)
                yield
                for kc in range(4, NKC):
                    nc.tensor.matmul(
                        v_ps[:], xt_sb[:, kc, :], wv_sb[:, kc, :],
                        start=False, stop=(kc == NKC - 1))
                # v -> vaug (strided per-head copy, leaves ones col intact)
                nc.scalar.copy(
                    vaug[:, mt, :, 0:HD],
                    v_ps[:].rearrange("p (h d) -> p h d", d=HD))
                yield

                tp = pp1.tile([128, 2, HPG // 2, 128], bf16, tag="ps1")
                transposes(0, tp, qT)
                yield
                transposes(1, tp, kT)
                state["mt_done"] = mt
                yield

        def attn_gen():
            for h, qw in [(h, qw) for qw in range(NQW - 1) for h in range(HPG)]:
                while state["mt_done"] < 4 * qw + 3:
                    yield "wait"
                po = (h % 2) * 64
                tr = h // 2
                n_kt = 4 * qw + 4
                pt = ptpool.tile([128, NKT, QW], bf16, tag="pt")
                for g in range(n_kt // 2):
                    st = stps.tile([128, 2, QW], f32, tag="st")
                    for j in range(2):
                        kt = 2 * g + j
                        d = kt - 4 * qw
                        col0 = d * 128 if d >= 0 else 0
                        nc.tensor.matmul(
                            st[:, j, col0:],
                            kT[po:po + 64, tr, kt * 128:(kt + 1) * 128],
                            qT[po:po + 64, tr,
                               qw * QW + col0:(qw + 1) * QW],
                            start=True, stop=True)
                    # batched exp over 2 k-tiles; stale PSUM cols left of
                    # the causal edge are exp'd but never read.  The 2nd
                    # diagonal pair only has cols 256+ live.
                    ec = 256 if 2 * g - 4 * qw >= 2 else 0
                    nc.scalar.activation(
                        pt[:, 2 * g:2 * g + 2, ec:], st[:, :, ec:],
                        AF.Exp, scale=1.0 / math.sqrt(HD))
                    for j in range(2):
                        kt = 2 * g + j
                        d = kt - 4 * qw
                        if d >= 0:
                            col0 = d * 128
                            nc.gpsimd.tensor_tensor(
                                pt[:, kt, col0:col0 + 128],
                                pt[:, kt, col0:col0 + 128], trim_sb[:],
                                op=MUL)
                    yield

                # transposed attn@v: one [128q, 65] chain per q-tile.
                # yT and ytp share one rotating PSUM bank (yT is dead once
                # the division has read it, so ytp may overwrite)
                yT = yend.tile([128, 4, HD + 1], f32, tag="ye")
                for i in range(4):
                    qt = 4 * qw + i
                    for kt in range(qt + 1):
                        nc.tensor.matmul(
                            yT[:, i, :],
                            pt[:, kt, i * 128:(i + 1) * 128],
                            vaug[:, kt, h, :],
                            start=(kt == 0), stop=(kt == qt))
                    if i == 1:
                        yield
                yield
                # divide by denominator (col 64, per-partition)
                rd = epil.tile([128, 4], f32, tag="rd")
                nc.vector.reciprocal(rd[:], yT[:, :, HD])
                rd_b = bass.AP(tensor=rd.tensor, offset=rd.offset,
                               ap=[rd.ap[0], [1, 4], [0, HD]])
                ysb = epil.tile([128, 4, HD], bf16, tag="ysb")
                nc.vector.tensor_tensor(ysb[:], yT[:, :, 0:HD], rd_b, op=MUL)
                yield
                # back to (ch x T) for the projection
                ytp = yend.tile([64, 4, 128], bf16, tag="ye")
                for i in range(4):
                    nc.tensor.transpose(ytp[:, i, :], ysb[:, i, :], ident[:])
                nc.vector.tensor_copy(
                    ybuf[po:po + 64, tr, qw * QW:(qw + 1) * QW]
                    .rearrange("p (i t) -> p i t", t=128),
                    ytp[:])
                state["win_done"][qw] += 1
                yield

            # ---- window 3, query-tile granular ----
            # w3's queries are produced last, so at window granularity its
            # whole exp stream would serialize after qkv production.  Each
            # (head, q-tile) only needs token tiles <= qt: its scores/exp/
            # attn@v run as soon as that tile lands; only qt=15 and the
            # batched epilogues remain in the tail.
            qw = NQW - 1
            for qt in range(4 * qw, NMT):
                while state["mt_done"] < qt:
                    yield "wait"
                for h in range(HPG):
                    po = (h % 2) * 64
                    tr = h // 2
                    pt3 = ptpool.tile([128, NKT, 128], bf16, tag="pt3")
                    for b in range(2):
                        kts = list(range(8 * b, min(8 * b + 8, qt + 1)))
                        st = stps.tile([128, 8, 128], f32, tag="st")
                        for i, kt in enumerate(kts):
                            nc.tensor.matmul(
                                st[:, i, :],
                                kT[po:po + 64, tr, kt * 128:(kt + 1) * 128],
                                qT[po:po + 64, tr, qt * 128:(qt + 1) * 128],
                                start=True, stop=True)
                        nc.scalar.activation(
                            pt3[:, 8 * b:8 * b + len(kts), :],
                            st[:, 0:len(kts), :],
                            AF.Exp, scale=1.0 / math.sqrt(HD))
                        if qt in kts:
                            nc.gpsimd.tensor_tensor(
                                pt3[:, qt, :], pt3[:, qt, :], trim_sb[:],
                                op=MUL)
                        yield
                    yT3 = yend.tile([128, HD + 1], f32, tag="ye")
                    for kt in range(qt + 1):
                        nc.tensor.matmul(
                            yT3[:], pt3[:, kt, :], vaug[:, kt, h, :],
                            start=(kt == 0), stop=(kt == qt))
                    # park the undivided result (and its denominator) in SBUF
                    nc.vector.tensor_copy(y3[:, h, qt - 4 * qw, :], yT3[:])
                    yield
            for h in range(HPG):
                po = (h % 2) * 64
                tr = h // 2
                rd = epil.tile([128, 4], f32, tag="rd")
                nc.vector.reciprocal(rd[:], y3[:, h, :, HD])
                rd_b = bass.AP(tensor=rd.tensor, offset=rd.offset,
                               ap=[rd.ap[0], [1, 4], [0, HD]])
                ysb = epil.tile([128, 4, HD], bf16, tag="ysb")
                nc.vector.tensor_tensor(ysb[:], y3[:, h, :, 0:HD], rd_b,
                                        op=MUL)
                ytp = yend.tile([64, 4, 128], bf16, tag="ye")
                for i in range(4):
                    nc.tensor.transpose(ytp[:, i, :], ysb[:, i, :], ident[:])
                nc.vector.tensor_copy(
                    ybuf[po:po + 64, tr, qw * QW:(qw + 1) * QW]
                    .rearrange("p (i t) -> p i t", t=128),
                    ytp[:])
                state["win_done"][qw] += 1
                yield

        def proj_gen():
            for mo, qw in [(mo, qw) for qw in range(NQW)
                           for mo in range(C // 128)]:
                while state["win_done"][qw] < HPG:
                    yield "wait"
                po_ps = pp1.tile([128, QW], f32, tag="ps1")
                for kc in range(CG // 128):
                    nc.tensor.matmul(
                        po_ps[:],
                        wp_sb[:, kc, mo * 128:(mo + 1) * 128],
                        ybuf[:, kc, qw * QW:(qw + 1) * QW],
                        start=(kc == 0), stop=(kc == CG // 128 - 1))
                ot = ostg.tile([128, QW], f32)
                nc.vector.tensor_copy(ot[:], po_ps[:])
                nc.sync.dma_start(
                    out[mo * 128:(mo + 1) * 128, qw * QW:(qw + 1) * QW],
                    ot[:])
                yield

        # weave: one p1/proj piece, then up to two attention pieces
        g_p1, g_at, g_pj = p1_gen(), attn_gen(), proj_gen()

        def step(g):
            if g is None:
                return None, False
            try:
                r = next(g)
                return g, r != "wait"
            except StopIteration:
                return None, False

        while g_p1 is not None or g_at is not None or g_pj is not None:
            progressed = False
            if g_p1 is not None:
                g_p1, ok = step(g_p1)
                progressed |= ok
            else:
                g_pj, ok = step(g_pj)
                progressed |= ok
            for _ in range(2):
                g_at, ok = step(g_at)
                progressed |= ok
                if g_at is None:
                    break
            if not progressed and g_p1 is None and g_at is None:
                # drain remaining proj
                while g_pj is not None:
                    g_pj, _ = step(g_pj)

    _split_excess_waits(nc, mybir)
    return nc


_NC_CACHE = {}


def _get_nc():
    if "nc" not in _NC_CACHE:
        _NC_CACHE["nc"] = _build_nc()
    return _NC_CACHE["nc"]


def _host_inputs(x, w_attn, w_proj):
    import ml_dtypes
    bf = ml_dtypes.bfloat16
    inv_freq = 1.0 / (10000.0 ** (np.arange(0, HD, 2, dtype=np.float32) / HD))
    t = np.arange(T, dtype=np.float32)
    freqs = np.outer(t, inv_freq)
    cos = np.cos(freqs).astype(bf)
    sin = np.sin(freqs).astype(bf)
    cosn = np.ascontiguousarray(cos.reshape(NMT, 128, HD // 2).transpose(1, 0, 2))
    sinn = np.ascontiguousarray(sin.reshape(NMT, 128, HD // 2).transpose(1, 0, 2))
    trim = np.triu(np.ones((128, 128), dtype=np.float32)).astype(bf)

    in_maps = []
    for b in range(B):
        xT = np.ascontiguousarray(x[b].T)  # (C, T)
        # [mt, ch-in-chunk, kc*128+tok]: one contiguous DMA per token tile,
        # partitions carry the contraction channels
        xt = np.ascontiguousarray(
            xT.reshape(NKC, 128, NMT, 128).transpose(2, 1, 0, 3)
        ).reshape(NMT, 128, NKC * 128).astype(bf)
        for hg in range(HG):
            qr = slice(hg * CG, (hg + 1) * CG)
            kr = slice(C + hg * CG, C + (hg + 1) * CG)
            vr = slice(2 * C + hg * CG, 2 * C + (hg + 1) * CG)
            wqk = np.ascontiguousarray(
                np.concatenate([w_attn[qr], w_attn[kr]], axis=0).T).astype(bf)
            wv = np.ascontiguousarray(w_attn[vr].T).astype(bf)
            wp = np.ascontiguousarray(w_proj[:, hg * CG:(hg + 1) * CG].T).astype(bf)
            in_maps.append({
                "xt": xt, "wqk": wqk, "wv": wv, "wp": wp,
                "cosn": cosn, "sinn": sinn, "trim": trim,
            })
    return in_maps


def kernel(x, w_attn, w_proj, _profile=False):
    from concourse.bass_utils import run_bass_kernel_spmd
    nc = _get_nc()
    in_maps = _host_inputs(
        np.asarray(x, dtype=np.float32),
        np.asarray(w_attn, dtype=np.float32),
        np.asarray(w_proj, dtype=np.float32))
    res = run_bass_kernel_spmd(nc, in_maps, core_ids=list(range(N_CORES)),
                               trace=_profile)
    out = np.empty((B, T, C), dtype=np.float32)
    for b in range(B):
        acc = res.results[2 * b]["o"] + res.results[2 * b + 1]["o"]
        out[b] = acc.T
    if _profile:
        return out, res
    return out


# revision 45
# speedup vs baseline: 1.0191x; 1.0191x over previous
"""Trainium2 Bass kernel for JointSelfAttention (B=4,T=2048,C=1024,H=16).

Sharding: 8 cores = 4 batches (data-parallel) x 2 head-groups of 8 heads
(tensor-parallel).  Each core computes qkv for its head group, qk-RMSNorm,
RoPE, causal attention, and a partial c_proj; the host sums the two partial
projections per batch and transposes back.

v3: software-pipelined emission.  Attention for query window w only needs
qkv of token tiles <= 4w+3, so attention units (one head x one window) are
interleaved into the qkv producer loop: the Act engine's exp stream (the
critical resource) runs concurrently with qkv GEMMs, norm/rope (DVE) and
projection.  PSUM is the scarce resource (8 banks): q/k/v share one rotating
1-bank accumulator (stats are computed per-tensor), score pairs use 2x2-bank
slots, attn output + transpose staging 1 bank each.  attn@v runs transposed
(stationary 128x128 softmax block, moving v[128x65]) so each block costs 65
PE rows and the softmax denominator lands per-partition.
"""

import math
import numpy as np
from contextlib import ExitStack

B, T, C, H, HD = 4, 2048, 1024, 16, 64
HG = 2              # head groups (tensor-parallel dim)
HPG = H // HG       # heads per group = 8
CG = HPG * HD       # channels per group = 512
N_CORES = B * HG
EPS = float(np.finfo(np.float32).eps)
QW = 512            # query window (free dim per attention block)
NQW = T // QW       # 4 windows
NKT = T // 128      # 16 k tiles
NMT = T // 128      # 16 m (token) tiles
NKC = C // 128      # 8 contraction tiles for qkv


def _split_excess_waits(nc, mybir, max_waits=1):
    """This container's walrus only encodes 1 sync-wait per instruction
    ("Too many sync wait commands" in CoreV3 codegen).  Move extra waits to
    preceding NoOps on the same engine."""
    for f in nc.m.functions:
        for bb in f.blocks:
            new_insts = []
            for inst in bb.instructions:
                si = inst.sync_info
                if si is not None and si.on_wait and len(si.on_wait) > max_waits:
                    waits = list(si.on_wait)
                    extra, keep = waits[:-max_waits], waits[-max_waits:]
                    for i in range(0, len(extra), max_waits):
                        nop = mybir.InstNoOp(
                            name=f"{inst.name}-ws{i}", ins=[], outs=[])
                        nop.engine = inst.engine
                        nop.sync_info = mybir.SyncInfo(
                            on_wait=extra[i:i + max_waits], on_update=[])
                        new_insts.append(nop)
                    inst.sync_info = mybir.SyncInfo(
                        on_wait=keep, on_update=list(si.on_update or []))
                new_insts.append(inst)
            bb.instructions.clear()
            bb.instructions.extend(new_insts)


def _build_nc():
    import concourse.bass as bass
    import concourse.tile as tile
    from concourse import mybir
    from concourse.masks import make_identity

    f32 = mybir.dt.float32
    bf16 = mybir.dt.bfloat16
    AF = mybir.ActivationFunctionType
    MUL = mybir.AluOpType.mult

    nc = bass.Bass("TRN2", debug=False, num_devices=N_CORES)

    xt = nc.dram_tensor("xt", [NMT, 128, NKC * 128], bf16, kind="ExternalInput").ap()
    wqk = nc.dram_tensor("wqk", [C, 2 * CG], bf16, kind="ExternalInput").ap()
    wv = nc.dram_tensor("wv", [C, CG], bf16, kind="ExternalInput").ap()
    wp = nc.dram_tensor("wp", [CG, C], bf16, kind="ExternalInput").ap()
    cosn = nc.dram_tensor("cosn", [128, NMT, HD // 2], bf16, kind="ExternalInput").ap()
    sinn = nc.dram_tensor("sinn", [128, NMT, HD // 2], bf16, kind="ExternalInput").ap()
    trim = nc.dram_tensor("trim", [128, 128], bf16, kind="ExternalInput").ap()
    out = nc.dram_tensor("o", [C, T], f32, kind="ExternalOutput").ap()

    with tile.TileContext(nc) as tc, ExitStack() as ctx:
        # ---- persistent buffers ----
        persist = ctx.enter_context(tc.tile_pool(name="persist", bufs=1))
        # q/k heads transposed: head h lives at partitions (h%2)*64..+64,
        # free slot h//2 -> [128, 4, T]
        qT = persist.tile([128, HPG // 2, T], bf16)
        kT = persist.tile([128, HPG // 2, T], bf16)
        vaug = persist.tile([128, NKT, HPG, HD + 1], bf16)  # v + ones col
        cos_sb = persist.tile([128, NMT, HD // 2], bf16)
        sin_sb = persist.tile([128, NMT, HD // 2], bf16)
        trim_sb = persist.tile([128, 128], bf16)
        ident = persist.tile([128, 128], bf16)
        eps_sb = persist.tile([128, 1], f32)
        ybuf = persist.tile([128, CG // 128, T], bf16)  # attn out (ch x T)
        wqk_sb = persist.tile([128, NKC, 2 * CG], bf16)
        wv_sb = persist.tile([128, NKC, CG], bf16)
        wp_sb = persist.tile([128, CG // 128, C], bf16)
        # window-3 undivided attn results (+denominator col), per (head, qt)
        y3 = persist.tile([128, HPG, 4, HD + 1], bf16)

        # weights ride the Activation HWDGE queue so the first x tile's DMA
        # (SP queue) isn't stuck behind them
        nc.scalar.dma_start(wqk_sb[:, :, 0:CG],
                            wqk[:, 0:CG].rearrange("(kc p) n -> p kc n", p=128))
        nc.scalar.dma_start(wqk_sb[:, :, CG:],
                            wqk[:, CG:].rearrange("(kc p) n -> p kc n", p=128))
        nc.scalar.dma_start(wv_sb[:], wv.rearrange("(kc p) n -> p kc n", p=128))
        nc.scalar.dma_start(cos_sb[:], cosn[:])
        nc.scalar.dma_start(sin_sb[:], sinn[:])
        nc.scalar.dma_start(trim_sb[:], trim[:])
        nc.scalar.dma_start(wp_sb[:], wp.rearrange("(kc p) n -> p kc n", p=128))
        make_identity(nc, ident[:])
        nc.vector.memset(eps_sb[:], EPS)
        nc.gpsimd.memset(
            vaug[:, :, :, HD:HD + 1].rearrange("p a b one -> p (a b one)"), 1.0)

        # ---- work generators, woven at sub-unit granularity ----
        # In-order engines suffer head-of-line blocking: inside an attention
        # unit the PE waits on exp (Act) between score groups.  Emitting qkv
        # GEMM chunks between score groups keeps both engines streaming.
        state = {"mt_done": -1, "win_done": [0] * NQW}

        xpool = ctx.enter_context(tc.tile_pool(name="xp", bufs=3))
        pp1 = ctx.enter_context(tc.tile_pool(name="pp1", bufs=3, space="PSUM"))
        stps = ctx.enter_context(tc.tile_pool(name="stps", bufs=2, space="PSUM"))
        yend = ctx.enter_context(tc.tile_pool(name="yend", bufs=1, space="PSUM"))
        ptpool = ctx.enter_context(tc.tile_pool(name="pt", bufs=3))
        epil = ctx.enter_context(tc.tile_pool(name="epil", bufs=3))
        work = ctx.enter_context(tc.tile_pool(name="work", bufs=3))
        stats = ctx.enter_context(tc.tile_pool(name="stats", bufs=2))
        ostg = ctx.enter_context(tc.tile_pool(name="ostg", bufs=3))

        def p1_gen():
            for mt in range(NMT):
                xt_sb = xpool.tile([128, NKC, 128], bf16, tag="xt")
                nc.sync.dma_start(
                    xt_sb[:].rearrange("p kc t -> p (kc t)"), xt[mt])

                qkn = work.tile([128, 2 * CG], bf16, tag="qkn")
                rr = stats.tile([128, 2 * HPG], f32, tag="rr")

                def stats_norm_rope(sec, s_ps):
                    # rms stats: rr = 1/sqrt(mean(x^2)+eps) per (token, head)
                    sq = work.tile([128, CG], bf16, tag="sq")
                    nc.scalar.activation(sq[:], s_ps[:], AF.Square)
                    ss = stats.tile([128, HPG], f32, tag="ss")
                    nc.vector.tensor_reduce(
                        ss[:], sq[:].rearrange("p (h d) -> p h d", d=HD),
                        axis=mybir.AxisListType.X, op=mybir.AluOpType.add)
                    nc.scalar.activation(rr[:, sec * HPG:(sec + 1) * HPG],
                                         ss[:], AF.Sqrt,
                                         bias=eps_sb[:], scale=1.0 / HD)
                    nc.vector.reciprocal(rr[:, sec * HPG:(sec + 1) * HPG],
                                         rr[:, sec * HPG:(sec + 1) * HPG])
                    # norm multiply doubles as PSUM->SBUF bf16 conversion
                    rr_b = bass.AP(
                        tensor=rr.tensor,
                        offset=rr.offset + sec * HPG,
                        ap=[rr.ap[0], [1, HPG], [0, HD]])
                    qs = qkn[:, sec * CG:(sec + 1) * CG]
                    nc.vector.tensor_tensor(
                        qs.rearrange("p (h d) -> p h d", d=HD),
                        s_ps[:].rearrange("p (h d) -> p h d", d=HD),
                        rr_b, op=MUL)
                    # rope in place: y1 = x1 c + x2 s ; y2 = x2 c - x1 s
                    base = qs.rearrange("p (h two d) -> p h two d",
                                        two=2, d=HD // 2)
                    x1, x2 = base[:, :, 0, :], base[:, :, 1, :]
                    cb = bass.AP(
                        tensor=cos_sb.tensor,
                        offset=cos_sb.offset + mt * (HD // 2),
                        ap=[cos_sb.ap[0], [0, HPG], [1, HD // 2]])
                    sb_ = bass.AP(
                        tensor=sin_sb.tensor,
                        offset=sin_sb.offset + mt * (HD // 2),
                        ap=[sin_sb.ap[0], [0, HPG], [1, HD // 2]])
                    t1 = work.tile([128, HPG, HD // 2], bf16, tag="rt1")
                    t2 = work.tile([128, HPG, HD // 2], bf16, tag="rt2")
                    t3 = work.tile([128, HPG, HD // 2], bf16, tag="rt3")
                    t4 = work.tile([128, HPG, HD // 2], bf16, tag="rt4")
                    nc.vector.tensor_tensor(t1[:], x1, cb, op=MUL)
                    nc.vector.tensor_tensor(t2[:], x2, sb_, op=MUL)
                    nc.vector.tensor_tensor(t3[:], x2, cb, op=MUL)
                    nc.vector.tensor_tensor(t4[:], x1, sb_, op=MUL)
                    nc.vector.tensor_add(x1, t1[:], t2[:])
                    nc.vector.tensor_sub(x2, t3[:], t4[:])

                def transposes(sec, tp, dstT):
                    # head pairs: [128 tok, (2h, 64d)] -> [128, 128]
                    # (partitions 0-63 = head 2j dims, 64-127 = head 2j+1)
                    for j in range(HPG // 2):
                        src = qkn[:, sec * CG + 2 * j * HD:
                                  sec * CG + (2 * j + 2) * HD]
                        nc.tensor.transpose(
                            tp[:, sec, j, :],
                            src.rearrange("p (two d) -> p two d", two=2),
                            ident[:])
                    nc.vector.tensor_copy(
                        dstT[:, :, mt * 128:(mt + 1) * 128], tp[:, sec])

                # q, k, v and the transpose staging rotate through three
                # 1-bank PSUM slots; stats/norm/rope drain while later
                # chunks and woven attention pieces run
                sps = []
                for sec in range(2):
                    s_ps = pp1.tile([128, CG], f32, tag="ps1")
                    sps.append(s_ps)
                    for kc in range(4):
                        nc.tensor.matmul(
                            s_ps[:], xt_sb[:, kc, :],
                            wqk_sb[:, kc, sec * CG:(sec + 1) * CG],
                            start=(kc == 0), stop=False)
                    yield
                    for kc in range(4, NKC):
                        nc.tensor.matmul(
                            s_ps[:], xt_sb[:, kc, :],
                            wqk_sb[:, kc, sec * CG:(sec + 1) * CG],
                            start=False, stop=(kc == NKC - 1))
                    stats_norm_rope(sec, s_ps)
                    yield

                v_ps = pp1.tile([128, CG], f32, tag="ps1")
                for kc in range(4):
                    nc.tensor.matmul(
                        v_ps[:], xt_sb[:, kc, :], wv_sb[:, kc, :],
                        start=(kc == 0), stop=False)
                yield
                for kc in range(4, NKC):
                    nc.tensor.matmul(
                        v_ps[:], xt_sb[:, kc, :], wv_sb[:, kc, :],
                        start=False, stop=(kc == NKC - 1))
                # v -> vaug (strided per-head copy, leaves ones col intact)
                nc.scalar.copy(
                    vaug[:, mt, :, 0:HD],
                    v_ps[:].rearrange("p (h d) -> p h d", d=HD))
                yield

                tp = pp1.tile([128, 2, HPG // 2, 128], bf16, tag="ps1")
                transposes(0, tp, qT)
                yield
                transposes(1, tp, kT)
                state["mt_done"] = mt
                yield

        def attn_gen():
            for h, qw in [(h, qw) for qw in range(NQW - 1) for h in range(HPG)]:
                while state["mt_done"] < 4 * qw + 3:
                    yield "wait"
                po = (h % 2) * 64
                tr = h // 2
                n_kt = 4 * qw + 4
                pt = ptpool.tile([128, NKT, QW], bf16, tag="pt")
                for g in range(n_kt // 2):
                    st = stps.tile([128, 2, QW], f32, tag="st")
                    for j in range(2):
                        kt = 2 * g + j
                        d = kt - 4 * qw
                        col0 = d * 128 if d >= 0 else 0
                        nc.tensor.matmul(
                            st[:, j, col0:],
                            kT[po:po + 64, tr, kt * 128:(kt + 1) * 128],
                            qT[po:po + 64, tr,
                               qw * QW + col0:(qw + 1) * QW],
                            start=True, stop=True)
                    # batched exp over 2 k-tiles; stale PSUM cols left of
                    # the causal edge are exp'd but never read.  The 2nd
                    # diagonal pair only has cols 256+ live.
                    ec = 256 if 2 * g - 4 * qw >= 2 else 0
                    nc.scalar.activation(
                        pt[:, 2 * g:2 * g + 2, ec:], st[:, :, ec:],
                        AF.Exp, scale=1.0 / math.sqrt(HD))
                    for j in range(2):
                        kt = 2 * g + j
                        d = kt - 4 * qw
                        if d >= 0:
                            col0 = d * 128
                            nc.gpsimd.tensor_tensor(
                                pt[:, kt, col0:col0 + 128],
                                pt[:, kt, col0:col0 + 128], trim_sb[:],
                                op=MUL)
                    yield

                # transposed attn@v: one [128q, 65] chain per q-tile.
                # yT and ytp share one rotating PSUM bank (yT is dead once
                # the division has read it, so ytp may overwrite)
                yT = yend.tile([128, 4, HD + 1], f32, tag="ye")
                for i in range(4):
                    qt = 4 * qw + i
                    for kt in range(qt + 1):
                        nc.tensor.matmul(
                            yT[:, i, :],
                            pt[:, kt, i * 128:(i + 1) * 128],
                            vaug[:, kt, h, :],
                            start=(kt == 0), stop=(kt == qt))
                    if i == 1:
                        yield
                yield
                # divide by denominator (col 64, per-partition)
                rd = epil.tile([128, 4], f32, tag="rd")
                nc.vector.reciprocal(rd[:], yT[:, :, HD])
                rd_b = bass.AP(tensor=rd.tensor, offset=rd.offset,
                               ap=[rd.ap[0], [1, 4], [0, HD]])
                ysb = epil.tile([128, 4, HD], bf16, tag="ysb")
                nc.vector.tensor_tensor(ysb[:], yT[:, :, 0:HD], rd_b, op=MUL)
                yield
                # back to (ch x T) for the projection
                ytp = yend.tile([64, 4, 128], bf16, tag="ye")
                for i in range(4):
                    nc.tensor.transpose(ytp[:, i, :], ysb[:, i, :], ident[:])
                nc.vector.tensor_copy(
                    ybuf[po:po + 64, tr, qw * QW:(qw + 1) * QW]
                    .rearrange("p (i t) -> p i t", t=128),
                    ytp[:])
                state["win_done"][qw] += 1
                yield

            # ---- window 3, query-tile granular ----
            # w3's queries are produced last, so at window granularity its
            # whole exp stream would serialize after qkv production.  Each
            # (head, q-tile) only needs token tiles <= qt: its scores/exp/
            # attn@v run as soon as that tile lands; only qt=15 and the
            # batched epilogues remain in the tail.
            qw = NQW - 1
            for qt in range(4 * qw, NMT):
                while state["mt_done"] < qt:
                    yield "wait"
                for h in range(HPG):
                    po = (h % 2) * 64
                    tr = h // 2
                    pt3 = ptpool.tile([128, NKT, 128], bf16, tag="pt3")
                    for b in range(2):
                        kts = list(range(8 * b, min(8 * b + 8, qt + 1)))
                        st = stps.tile([128, 8, 128], f32, tag="st")
                        for i, kt in enumerate(kts):
                            nc.tensor.matmul(
                                st[:, i, :],
                                kT[po:po + 64, tr, kt * 128:(kt + 1) * 128],
                                qT[po:po + 64, tr, qt * 128:(qt + 1) * 128],
                                start=True, stop=True)
                        nc.scalar.activation(
                            pt3[:, 8 * b:8 * b + len(kts), :],
                            st[:, 0:len(kts), :],
                            AF.Exp, scale=1.0 / math.sqrt(HD))
                        if qt in kts:
                            nc.gpsimd.tensor_tensor(
                                pt3[:, qt, :], pt3[:, qt, :], trim_sb[:],
                                op=MUL)
                        yield
                    yT3 = yend.tile([128, HD + 1], f32, tag="ye")
                    for kt in range(qt + 1):
                        nc.tensor.matmul(
                            yT3[:], pt3[:, kt, :], vaug[:, kt, h, :],
                            start=(kt == 0), stop=(kt == qt))
                    # park the undivided result (and its denominator) in SBUF
                    nc.vector.tensor_copy(y3[:, h, qt - 4 * qw, :], yT3[:])
                    yield
            for h in range(HPG):
                po = (h % 2) * 64
                tr = h // 2
                rd = epil.tile([128, 4], f32, tag="rd")
                nc.vector.reciprocal(rd[:], y3[:, h, :, HD])
                rd_b = bass.AP(tensor=rd.tensor, offset=rd.offset,
                               ap=[rd.ap[0], [1, 4], [0, HD]])
                ysb = epil.tile([128, 4, HD], bf16, tag="ysb")
                nc.vector.tensor_tensor(ysb[:], y3[:, h, :, 0:HD], rd_b,
                                        op=MUL)
                ytp = yend.tile([64, 4, 128], bf16, tag="ye")
                for i in range(4):
                    nc.tensor.transpose(ytp[:, i, :], ysb[:, i, :], ident[:])
                nc.vector.tensor_copy(
                    ybuf[po:po + 64, tr, qw * QW:(qw + 1) * QW]
                    .rearrange("p (i t) -> p i t", t=128),
                    ytp[:])
                state["win_done"][qw] += 1
                yield

        def proj_gen():
            for mo, qw in [(mo, qw) for qw in range(NQW)
                           for mo in range(C // 128)]:
                while state["win_done"][qw] < HPG:
                    yield "wait"
                po_ps = pp1.tile([128, QW], f32, tag="ps1")
                for kc in range(CG // 128):
                    nc.tensor.matmul(
                        po_ps[:],
                        wp_sb[:, kc, mo * 128:(mo + 1) * 128],
                        ybuf[:, kc, qw * QW:(qw + 1) * QW],
                        start=(kc == 0), stop=(kc == CG // 128 - 1))
                ot = ostg.tile([128, QW], f32)
                nc.vector.tensor_copy(ot[:], po_ps[:])
                nc.sync.dma_start(
                    out[mo * 128:(mo + 1) * 128, qw * QW:(qw + 1) * QW],
                    ot[:])
                yield

        # weave: one p1/proj piece, then up to two attention pieces
        g_p1, g_at, g_pj = p1_gen(), attn_gen(), proj_gen()

        def step(g):
            if g is None:
                return None, False
            try:
                r = next(g)
                return g, r != "wait"
            except StopIteration:
                return None, False

        while g_p1 is not None or g_at is not None or g_pj is not None:
            progressed = False
            if g_p1 is not None:
                g_p1, ok = step(g_p1)
                progressed |= ok
            else:
                g_pj, ok = step(g_pj)
                progressed |= ok
            for _ in range(2):
                g_at, ok = step(g_at)
                progressed |= ok
                if g_at is None:
                    break
            if not progressed and g_p1 is None and g_at is None:
                # drain remaining proj
                while g_pj is not None:
                    g_pj, _ = step(g_pj)

    _split_excess_waits(nc, mybir)
    return nc


_NC_CACHE = {}


def _get_nc():
    if "nc" not in _NC_CACHE:
        _NC_CACHE["nc"] = _build_nc()
    return _NC_CACHE["nc"]


def _host_inputs(x, w_attn, w_proj):
    import ml_dtypes
    bf = ml_dtypes.bfloat16
    inv_freq = 1.0 / (10000.0 ** (np.arange(0, HD, 2, dtype=np.float32) / HD))
    t = np.arange(T, dtype=np.float32)
    freqs = np.outer(t, inv_freq)
    cos = np.cos(freqs).astype(bf)
    sin = np.sin(freqs).astype(bf)
    cosn = np.ascontiguousarray(cos.reshape(NMT, 128, HD // 2).transpose(1, 0, 2))
    sinn = np.ascontiguousarray(sin.reshape(NMT, 128, HD // 2).transpose(1, 0, 2))
    trim = np.triu(np.ones((128, 128), dtype=np.float32)).astype(bf)

    in_maps = []
    for b in range(B):
        xT = np.ascontiguousarray(x[b].T)  # (C, T)
        # [mt, ch-in-chunk, kc*128+tok]: one contiguous DMA per token tile,
        # partitions carry the contraction channels
        xt = np.ascontiguousarray(
            xT.reshape(NKC, 128, NMT, 128).transpose(2, 1, 0, 3)
        ).reshape(NMT, 128, NKC * 128).astype(bf)
        for hg in range(HG):
            qr = slice(hg * CG, (hg + 1) * CG)
            kr = slice(C + hg * CG, C + (hg + 1) * CG)
            vr = slice(2 * C + hg * CG, 2 * C + (hg + 1) * CG)
            wqk = np.ascontiguousarray(
                np.concatenate([w_attn[qr], w_attn[kr]], axis=0).T).astype(bf)
            wv = np.ascontiguousarray(w_attn[vr].T).astype(bf)
            wp = np.ascontiguousarray(w_proj[:, hg * CG:(hg + 1) * CG].T).astype(bf)
            in_maps.append({
                "xt": xt, "wqk": wqk, "wv": wv, "wp": wp,
                "cosn": cosn, "sinn": sinn, "trim": trim,
            })
    return in_maps


def kernel(x, w_attn, w_proj, _profile=False):
    from concourse.bass_utils import run_bass_kernel_spmd
    nc = _get_nc()
    in_maps = _host_inputs(
        np.asarray(x, dtype=np.float32),
        np.asarray(w_attn, dtype=np.float32),
        np.asarray(w_proj, dtype=np.float32))
    res = run_bass_kernel_spmd(nc, in_maps, core_ids=list(range(N_CORES)),
                               trace=_profile)
    out = np.empty((B, T, C), dtype=np.float32)
    for b in range(B):
        acc = res.results[2 * b]["o"] + res.results[2 * b + 1]["o"]
        out[b] = acc.T
    if _profile:
        return out, res
    return out
